# revision 10
# baseline (speedup 1.0000x reference)
"""KNN loss kernel for Trainium2 (Bass/Tile), data-parallel over batch.

Math: for each batch b (one per NeuronCore), compute
  w_ij = R^2 - ||pc_i - pc_j||^2 = 2*pc_i.pc_j - ||pc_j||^2 + (R^2 - ||pc_i||^2)
via a single K=5 augmented matmul (PE), so the top-16 largest w per row are the
16 nearest neighbors and w>0 <=> in-radius.

Top-16 extraction (per 128-row block) avoids full-row max_index scans by
packing the column id into the low 12 mantissa bits of w:
  packed = (w_bits & 0xFFFFF000) | col_id
which preserves float ordering to ~2^-11 relative (w is radius-shifted so all
relevant values live in binades <= 2^-4 => absolute quantization <= 3e-5 on
squared distances; boundary ties just pick an almost-equidistant neighbor).
Per 512-col slice a single DVE max8 yields that slice's top-8 packed values;
the global top-16 is then found among the 8x8=64 slice winners with one
max8 + match_replace + max8 on a 64-wide tile. Column ids come back via a
bitwise AND. Out-of-radius slots (w<=0, which sort below any in-radius value)
are replaced with the row's own index => zero flow diff, as in the reference.

The kernel outputs the [4096,16] neighbor index matrix per core; the host
does the O(N*K) flow gather + L1 + mean.
"""

from contextlib import ExitStack

import numpy as np

import concourse.bacc as bacc
import concourse.mybir as mybir
import concourse.tile as tile
from concourse.bass_utils import run_bass_kernel_spmd

B = 8
N = 4096
K = 16
RADIUS = 0.25
R2 = RADIUS * RADIUS
BLK = 128
NBLK = N // BLK  # 32
SLC = 512
NSLC = N // SLC  # 8
CHUNK = 2048  # pack granularity (4 PSUM banks)
NCHUNK = N // CHUNK
F32 = mybir.dt.float32
U32 = mybir.dt.uint32
U8 = mybir.dt.uint8


def _build_program():
    nc = bacc.Bacc(
        "TRN2",
        target_bir_lowering=False,
        debug=False,
        num_devices=B,
    )
    lhsT_d = nc.dram_tensor("lhsT", [5, N], F32, kind="ExternalInput").ap()
    rhs_d = nc.dram_tensor("rhs", [5, N], F32, kind="ExternalInput").ap()
    rowid_d = nc.dram_tensor("rowid", [BLK, NBLK], U32, kind="ExternalInput").ap()
    colid_d = nc.dram_tensor("colid", [BLK, N], U32, kind="ExternalInput").ap()
    consts_d = nc.dram_tensor("consts", [BLK, 2], U32, kind="ExternalInput").ap()
    idx_out_d = nc.dram_tensor("idx_out", [N, K], U32, kind="ExternalOutput").ap()

    with tile.TileContext(nc) as tc:
        with ExitStack() as ctx:
            const = ctx.enter_context(tc.tile_pool(name="const", bufs=1))
            psum = ctx.enter_context(tc.tile_pool(name="psum", bufs=2, space="PSUM"))
            wpool = ctx.enter_context(tc.tile_pool(name="w", bufs=2))
            small = ctx.enter_context(tc.tile_pool(name="small", bufs=6))

            lhsT = const.tile([5, N], F32)
            nc.sync.dma_start(lhsT[:], lhsT_d[:])
            rhs = const.tile([5, N], F32)
            nc.sync.dma_start(rhs[:], rhs_d[:])
            rowid = const.tile([BLK, NBLK], U32)
            nc.sync.dma_start(rowid[:], rowid_d[:])
            colid = const.tile([BLK, N], U32)
            nc.sync.dma_start(colid[:], colid_d[:])
            consts = const.tile([BLK, 2], U32)
            nc.sync.dma_start(consts[:], consts_d[:])
            mask_hi = consts[:, 0:1]  # 0xFFFFF000 per partition
            mask_lo = consts[:, 1:2]  # 0x00000FFF per partition

            for I in range(NBLK):
                packed = wpool.tile([BLK, N], F32)
                for ch in range(NCHUNK):
                    ps = psum.tile([BLK, CHUNK], F32)
                    for h in range(CHUNK // SLC):
                        c = ch * (CHUNK // SLC) + h
                        nc.tensor.matmul(
                            ps[:, h * SLC : (h + 1) * SLC],
                            lhsT[:, I * BLK : (I + 1) * BLK],
                            rhs[:, c * SLC : (c + 1) * SLC],
                            start=True,
                            stop=True,
                        )
                    # packed = (w & 0xFFFFF000) | colid   (DVE, PSUM -> SBUF)
                    pk = packed[:, ch * CHUNK : (ch + 1) * CHUNK].bitcast(U32)
                    cid = colid[:, ch * CHUNK : (ch + 1) * CHUNK]
                    nc.vector.scalar_tensor_tensor(
                        pk,
                        ps[:].bitcast(U32),
                        mask_hi,
                        cid,
                        op0=mybir.AluOpType.bitwise_and,
                        op1=mybir.AluOpType.bitwise_or,
                    )

                cand = small.tile([BLK, 8 * NSLC], F32, tag="cand")
                for c in range(NSLC):
                    nc.vector.max(
                        cand[:, c * 8 : (c + 1) * 8],
                        packed[:, c * SLC : (c + 1) * SLC],
                    )
                winners = small.tile([BLK, K], F32, tag="winners")
                nc.vector.max(winners[:, 0:8], cand[:])
                nc.vector.match_replace(cand[:], winners[:, 0:8], cand[:], -1e30)
                nc.vector.max(winners[:, 8:16], cand[:])

                iidx = small.tile([BLK, K], U32, tag="iidx")
                nc.vector.tensor_scalar(
                    iidx[:],
                    winners[:].bitcast(U32),
                    mask_lo,
                    scalar2=None,
                    op0=mybir.AluOpType.bitwise_and,
                )
                sel = small.tile([BLK, K], U8, tag="sel")
                nc.vector.tensor_scalar(
                    sel[:], winners[:], 1e-30, scalar2=None, op0=mybir.AluOpType.is_gt
                )
                out_t = small.tile([BLK, K], U32, tag="out")
                nc.vector.tensor_copy(
                    out_t[:], rowid[:, I : I + 1].to_broadcast([BLK, K])
                )
                nc.vector.copy_predicated(out_t[:], sel[:], iidx[:])
                nc.sync.dma_start(idx_out_d[I * BLK : (I + 1) * BLK, :], out_t[:])
    nc.compile()
    return nc


_NC_CACHE = {}


def _get_program():
    if "nc" not in _NC_CACHE:
        _NC_CACHE["nc"] = _build_program()
    return _NC_CACHE["nc"]


def run_device(pc: np.ndarray, trace: bool = False):
    """Run the 8-core SPMD kernel; returns (list of per-core idx [N,K] uint32,
    BassKernelResults)."""
    pc = np.asarray(pc, dtype=np.float32)
    sq = (pc.astype(np.float32) ** 2).sum(-1)  # [B, N]
    ones = np.ones((1, N), np.float32)
    rowid = (
        np.arange(N, dtype=np.uint32).reshape(NBLK, BLK).T
    ).copy()  # rowid[p, I] = I*BLK + p
    colid = np.broadcast_to(np.arange(N, dtype=np.uint32)[None, :], (BLK, N)).copy()
    consts = np.empty((BLK, 2), np.uint32)
    consts[:, 0] = np.uint32(0xFFFFF000)
    consts[:, 1] = np.uint32(0x00000FFF)
    in_maps = []
    for b in range(B):
        lhsT = np.concatenate(
            [pc[b].T, ones, (R2 - sq[b])[None, :]], axis=0
        ).astype(np.float32)
        rhs = np.concatenate(
            [2.0 * pc[b].T, -sq[b][None, :], ones], axis=0
        ).astype(np.float32)
        in_maps.append(
            {
                "lhsT": np.ascontiguousarray(lhsT),
                "rhs": np.ascontiguousarray(rhs),
                "rowid": rowid,
                "colid": colid,
                "consts": consts,
            }
        )
    nc = _get_program()
    res = run_bass_kernel_spmd(
        nc, in_maps, core_ids=list(range(B)), trace=trace
    )
    idxs = [res.results[b]["idx_out"] for b in range(B)]
    return idxs, res


def kernel(pc: np.ndarray, flow: np.ndarray) -> np.ndarray:
    pc = np.asarray(pc, dtype=np.float32)
    flow = np.asarray(flow, dtype=np.float32)
    idxs, _ = run_device(pc)
    total = 0.0
    for b in range(B):
        idx = idxs[b].astype(np.int64)  # [N, K]
        nn_flow = flow[b][idx]  # [N, K, 3]
        diff = flow[b][:, None, :] - nn_flow
        total += float(np.abs(diff).sum(dtype=np.float64))
    return np.float32(total / (B * N * K))
